# revision 12
# baseline (speedup 1.0000x reference)
"""BitLinear 2-bit quantized linear layer on 8 TRN2 NeuronCores.

Math: reference computes
    a      = clip(max|x| over last dim, EPS)
    out    = ((x/a) @ W_deq^T) * (a*scale) + bias,  W_deq = QUANT_LEVELS[codes]
The per-row absmax normalization cancels exactly (division by `a` then
multiplication by the same `a`), so out == (x @ W_deq^T) * scale + bias.
QUANT_LEVELS[c] = c - 1.5, so W_deq (and W_deq*scale for scale=1) is exactly
representable in bf16. We therefore run a plain bf16 matmul with fp32 PSUM
accumulation and a bias epilogue.

Sharding: data-parallel over the 8192 = 4*2048 (batch*seq) rows; each of the
8 cores computes a [1024, 4096] slice of the output with the full weight.
Host pre-transposes both operands so the device only does DMA + matmul:
  xT [K=4096, M=1024] bf16 per core, wT [K=4096, N=4096] bf16 replicated.
"""

import numpy as np
import ml_dtypes

import concourse.mybir as mybir
from concourse import bacc
from concourse.tile import TileContext
from concourse.bass_utils import run_bass_kernel_spmd

N_CORES = 8
B, S, D_IN, D_OUT = 4, 2048, 4096, 4096
M_TOTAL = B * S              # 8192 rows
M = M_TOTAL // N_CORES       # 1024 rows per core
K = D_IN
N = D_OUT
P = 128                      # partitions
KI = K // P                  # 32 k-tiles
NF = 512                     # psum free dim (one PSUM bank of fp32)
NI = N // NF                 # 8 n-chunks
MI = M // P                  # 8 m-tiles

BF16 = mybir.dt.bfloat16
F32 = mybir.dt.float32


def build(m=M, k=K, n=N):
    ki, mi_n, ni_n = k // P, m // P, n // NF
    nc = bacc.Bacc(enable_partition_id=False)
    xT = nc.declare_dram_parameter("xT", [k, m], BF16, isOutput=False)
    wT = nc.declare_dram_parameter("wT", [k, n], BF16, isOutput=False)
    bias = nc.declare_dram_parameter("bias", [P, n], F32, isOutput=False)
    out = nc.declare_dram_parameter("out", [m, n], F32, isOutput=True)

    xT3 = xT[:].rearrange("(a p) m -> p a m", p=P)   # [128, ki, m]
    wT3 = wT[:].rearrange("(a p) n -> p a n", p=P)   # [128, ki, n]

    with TileContext(nc) as tc:
        with (
            tc.tile_pool(name="xpool", bufs=1) as xpool,
            tc.tile_pool(name="bpool", bufs=1) as bpool,
            tc.tile_pool(name="wpool", bufs=2) as wpool,
            tc.tile_pool(name="opool", bufs=6) as opool,
            tc.tile_pool(name="ppool", bufs=8, space="PSUM") as ppool,
        ):
            # x is resident for the whole kernel; the first W chunk and x are
            # loaded interleaved in ki-order pieces so ni=0 matmuls can start
            # after ~1.5 MiB instead of the full 12 MiB. x goes through the
            # ACT DGE ring and w through the SP ring so descriptor generation
            # for the two streams runs in parallel.
            xt = xpool.tile([P, ki, m], BF16, name="xt")
            wg = 8 if ki % 8 == 0 else 1
            kj = ki // wg
            wt0 = wpool.tile([P, ki, NF], BF16, name="wt")
            if wg > 1:
                # smaller leading pieces so the first matmuls unblock sooner
                chunk_sizes = [kj // 2, kj // 2, kj // 2, kj // 2] + [kj] * (wg - 2)
            else:
                chunk_sizes = [kj]
            pos = 0
            for cs in chunk_sizes:
                sl = slice(pos, pos + cs)
                nc.scalar.dma_start(out=xt[:, sl, :], in_=xT3[:, sl, :])
                nc.sync.dma_start(out=wt0[:, sl, :], in_=wT3[:, sl, 0:NF])
                pos += cs
            bias_sb = bpool.tile([P, n], F32, name="bias_sb")
            nc.scalar.dma_start(out=bias_sb[:], in_=bias[:])

            # PE warmup: dummy matmuls on zeroed tiles keep the PE busy while
            # the first data chunks stream in, so the HAM clock-gate reaches
            # 2.4 GHz before the real accumulation starts (saves the ~10 us
            # cold-clock window). Results land in a psum bank that the real
            # ni=0 group overwrites (start=True resets the bank).
            warm_l = bpool.tile([P, P], BF16, name="warm_l")
            warm_r = bpool.tile([P, NF], BF16, name="warm_r")
            nc.vector.memset(warm_l[:], 0.0)
            nc.vector.memset(warm_r[:], 0.0)

            def epilogue(ps, mi, nsl):
                ot = opool.tile([P, NF], F32, name="ot")
                nc.vector.tensor_add(out=ot[:], in0=ps[:], in1=bias_sb[:, nsl])
                nc.sync.dma_start(out=out[mi * P:(mi + 1) * P, nsl], in_=ot[:])

            wt = wt0
            for ni in range(ni_n):
                nsl = slice(ni * NF, (ni + 1) * NF)
                wt_next = None
                if ni + 1 < ni_n:
                    wt_next = wpool.tile([P, ki, NF], BF16, name="wt")
                if ni == 0:
                    # ki-chunk-major over all 8 psum banks: accumulate into
                    # every mi's bank as each ki piece of x/w arrives, so PE
                    # rides right behind the startup DMA stream.
                    pss = [ppool.tile([P, NF], F32, name="ps") for _ in range(mi_n)]
                    for _ in range(22):
                        nc.tensor.matmul(
                            pss[mi_n - 1][:], lhsT=warm_l[:], rhs=warm_r[:],
                            start=True, stop=True,
                        )
                    for g in range(wg):
                        for mi in range(mi_n):
                            for kk in range(g * kj, (g + 1) * kj):
                                nc.tensor.matmul(
                                    pss[mi][:],
                                    lhsT=xt[:, kk, mi * P:(mi + 1) * P],
                                    rhs=wt[:, kk, :],
                                    start=(kk == 0),
                                    stop=(kk == ki - 1),
                                )
                        # delay the ni=1 weight prefetch until the startup
                        # stream is done so they don't race for HBM bandwidth
                        if g == wg - 1 and wt_next is not None:
                            for g2 in range(wg):
                                sl = slice(g2 * kj, (g2 + 1) * kj)
                                nc.sync.dma_start(
                                    out=wt_next[:, sl, :],
                                    in_=wT3[:, sl, NF:2 * NF],
                                )
                    for mi in range(mi_n):
                        epilogue(pss[mi], mi, nsl)
                else:
                    for mi in range(mi_n):
                        ps = ppool.tile([P, NF], F32, name="ps")
                        for kk in range(ki):
                            nc.tensor.matmul(
                                ps[:],
                                lhsT=xt[:, kk, mi * P:(mi + 1) * P],
                                rhs=wt[:, kk, :],
                                start=(kk == 0),
                                stop=(kk == ki - 1),
                            )
                        epilogue(ps, mi, nsl)
                        # spread next-chunk weight DMA issue across the phase
                        if wt_next is not None and mi < wg:
                            sl = slice(mi * kj, (mi + 1) * kj)
                            nc.sync.dma_start(
                                out=wt_next[:, sl, :],
                                in_=wT3[:, sl, (ni + 1) * NF:(ni + 2) * NF],
                            )
                wt = wt_next
    nc.finalize()
    return nc


_NC = None


def _get_nc():
    global _NC
    if _NC is None:
        _NC = build()
    return _NC


def make_in_maps(x, weight_2bit, weight_scale, bias):
    x = np.asarray(x)
    codes = np.asarray(weight_2bit)
    ws = np.float32(np.asarray(weight_scale).reshape(-1)[0])
    b = np.asarray(bias).astype(np.float32)

    w_f = (codes.astype(np.float32) - np.float32(1.5)) * ws      # [N, K]
    wT = np.ascontiguousarray(w_f.T.astype(ml_dtypes.bfloat16))  # [K, N]
    bias_rep = np.ascontiguousarray(np.broadcast_to(b, (P, N)))

    x2 = x.reshape(M_TOTAL, K).astype(ml_dtypes.bfloat16)
    in_maps = []
    for c in range(N_CORES):
        xTc = np.ascontiguousarray(x2[c * M:(c + 1) * M].T)      # [K, M]
        in_maps.append({"xT": xTc, "wT": wT, "bias": bias_rep})
    return in_maps


def run(in_maps, trace=False, **kw):
    return run_bass_kernel_spmd(
        _get_nc(), in_maps, list(range(N_CORES)), trace=trace, **kw
    )


def kernel(x, weight_2bit, weight_scale, bias):
    res = run(make_in_maps(x, weight_2bit, weight_scale, bias))
    out = np.concatenate([r["out"] for r in res.results], axis=0)
    return np.ascontiguousarray(out.reshape(B, S, N))


# revision 13
# speedup vs baseline: 1.0061x; 1.0061x over previous
"""BitLinear 2-bit quantized linear layer on 8 TRN2 NeuronCores.

Math: reference computes
    a      = clip(max|x| over last dim, EPS)
    out    = ((x/a) @ W_deq^T) * (a*scale) + bias,  W_deq = QUANT_LEVELS[codes]
The per-row absmax normalization cancels exactly (division by `a` then
multiplication by the same `a`), so out == (x @ W_deq^T) * scale + bias.
QUANT_LEVELS[c] = c - 1.5, so W_deq (and W_deq*scale for scale=1) is exactly
representable in bf16. We therefore run a plain bf16 matmul with fp32 PSUM
accumulation and a bias epilogue.

Sharding: data-parallel over the 8192 = 4*2048 (batch*seq) rows; each of the
8 cores computes a [1024, 4096] slice of the output with the full weight.
Host pre-transposes both operands so the device only does DMA + matmul:
  xT [K=4096, M=1024] bf16 per core, wT [K=4096, N=4096] bf16 replicated.
"""

import numpy as np
import ml_dtypes

import concourse.mybir as mybir
from concourse import bacc
from concourse.tile import TileContext
from concourse.bass_utils import run_bass_kernel_spmd

N_CORES = 8
B, S, D_IN, D_OUT = 4, 2048, 4096, 4096
M_TOTAL = B * S              # 8192 rows
M = M_TOTAL // N_CORES       # 1024 rows per core
K = D_IN
N = D_OUT
P = 128                      # partitions
KI = K // P                  # 32 k-tiles
NF = 512                     # psum free dim (one PSUM bank of fp32)
NI = N // NF                 # 8 n-chunks
MI = M // P                  # 8 m-tiles

BF16 = mybir.dt.bfloat16
F32 = mybir.dt.float32


def build(m=M, k=K, n=N):
    ki, mi_n, ni_n = k // P, m // P, n // NF
    nc = bacc.Bacc()
    xT = nc.declare_dram_parameter("xT", [k, m], BF16, isOutput=False)
    wT = nc.declare_dram_parameter("wT", [k, n], BF16, isOutput=False)
    bias = nc.declare_dram_parameter("bias", [P, n], F32, isOutput=False)
    out = nc.declare_dram_parameter("out", [m, n], F32, isOutput=True)

    xT3 = xT[:].rearrange("(a p) m -> p a m", p=P)   # [128, ki, m]
    wT3 = wT[:].rearrange("(a p) n -> p a n", p=P)   # [128, ki, n]

    with TileContext(nc) as tc:
        with (
            tc.tile_pool(name="xpool", bufs=1) as xpool,
            tc.tile_pool(name="bpool", bufs=1) as bpool,
            tc.tile_pool(name="wpool", bufs=2) as wpool,
            tc.tile_pool(name="opool", bufs=6) as opool,
            tc.tile_pool(name="ppool", bufs=8, space="PSUM") as ppool,
        ):
            # x is resident for the whole kernel; the first W chunk and x are
            # loaded interleaved in ki-order pieces so ni=0 matmuls can start
            # after ~1.5 MiB instead of the full 12 MiB. x goes through the
            # ACT DGE ring and w through the SP ring so descriptor generation
            # for the two streams runs in parallel.
            xt = xpool.tile([P, ki, m], BF16, name="xt")
            wg = 8 if ki % 8 == 0 else 1
            kj = ki // wg
            wt0 = wpool.tile([P, ki, NF], BF16, name="wt")
            if wg > 1:
                # smaller leading pieces so the first matmuls unblock sooner
                chunk_sizes = [kj // 2, kj // 2, kj // 2, kj // 2] + [kj] * (wg - 2)
            else:
                chunk_sizes = [kj]
            pos = 0
            for cs in chunk_sizes:
                sl = slice(pos, pos + cs)
                nc.scalar.dma_start(out=xt[:, sl, :], in_=xT3[:, sl, :])
                nc.sync.dma_start(out=wt0[:, sl, :], in_=wT3[:, sl, 0:NF])
                pos += cs
            bias_sb = bpool.tile([P, n], F32, name="bias_sb")
            nc.scalar.dma_start(out=bias_sb[:], in_=bias[:])

            # PE warmup: dummy matmuls on zeroed tiles keep the PE busy while
            # the first data chunks stream in, so the HAM clock-gate reaches
            # 2.4 GHz before the real accumulation starts (saves the ~10 us
            # cold-clock window). Results land in a psum bank that the real
            # ni=0 group overwrites (start=True resets the bank).
            warm_l = bpool.tile([P, P], BF16, name="warm_l")
            warm_r = bpool.tile([P, NF], BF16, name="warm_r")
            nc.vector.memset(warm_l[:], 0.0)
            nc.vector.memset(warm_r[:], 0.0)

            def epilogue(ps, mi, nsl):
                ot = opool.tile([P, NF], F32, name="ot")
                nc.vector.tensor_add(out=ot[:], in0=ps[:], in1=bias_sb[:, nsl])
                nc.sync.dma_start(out=out[mi * P:(mi + 1) * P, nsl], in_=ot[:])

            wt = wt0
            for ni in range(ni_n):
                nsl = slice(ni * NF, (ni + 1) * NF)
                wt_next = None
                if ni + 1 < ni_n:
                    wt_next = wpool.tile([P, ki, NF], BF16, name="wt")
                if ni == 0:
                    # ki-chunk-major over all 8 psum banks: accumulate into
                    # every mi's bank as each ki piece of x/w arrives, so PE
                    # rides right behind the startup DMA stream.
                    pss = [ppool.tile([P, NF], F32, name="ps") for _ in range(mi_n)]
                    for _ in range(22):
                        nc.tensor.matmul(
                            pss[mi_n - 1][:], lhsT=warm_l[:], rhs=warm_r[:],
                            start=True, stop=True,
                        )
                    for g in range(wg):
                        for mi in range(mi_n):
                            for kk in range(g * kj, (g + 1) * kj):
                                nc.tensor.matmul(
                                    pss[mi][:],
                                    lhsT=xt[:, kk, mi * P:(mi + 1) * P],
                                    rhs=wt[:, kk, :],
                                    start=(kk == 0),
                                    stop=(kk == ki - 1),
                                )
                        # delay the ni=1 weight prefetch until the startup
                        # stream is done so they don't race for HBM bandwidth
                        if g == wg - 1 and wt_next is not None:
                            for g2 in range(wg):
                                sl = slice(g2 * kj, (g2 + 1) * kj)
                                nc.sync.dma_start(
                                    out=wt_next[:, sl, :],
                                    in_=wT3[:, sl, NF:2 * NF],
                                )
                    for mi in range(mi_n):
                        epilogue(pss[mi], mi, nsl)
                else:
                    for mi in range(mi_n):
                        ps = ppool.tile([P, NF], F32, name="ps")
                        for kk in range(ki):
                            nc.tensor.matmul(
                                ps[:],
                                lhsT=xt[:, kk, mi * P:(mi + 1) * P],
                                rhs=wt[:, kk, :],
                                start=(kk == 0),
                                stop=(kk == ki - 1),
                            )
                        epilogue(ps, mi, nsl)
                        # spread next-chunk weight DMA issue across the phase
                        if wt_next is not None and mi < wg:
                            sl = slice(mi * kj, (mi + 1) * kj)
                            nc.sync.dma_start(
                                out=wt_next[:, sl, :],
                                in_=wT3[:, sl, (ni + 1) * NF:(ni + 2) * NF],
                            )
                wt = wt_next
    nc.finalize()
    return nc


_NC = None


def _get_nc():
    global _NC
    if _NC is None:
        _NC = build()
    return _NC


def make_in_maps(x, weight_2bit, weight_scale, bias):
    x = np.asarray(x)
    codes = np.asarray(weight_2bit)
    ws = np.float32(np.asarray(weight_scale).reshape(-1)[0])
    b = np.asarray(bias).astype(np.float32)

    w_f = (codes.astype(np.float32) - np.float32(1.5)) * ws      # [N, K]
    wT = np.ascontiguousarray(w_f.T.astype(ml_dtypes.bfloat16))  # [K, N]
    bias_rep = np.ascontiguousarray(np.broadcast_to(b, (P, N)))

    x2 = x.reshape(M_TOTAL, K).astype(ml_dtypes.bfloat16)
    in_maps = []
    for c in range(N_CORES):
        xTc = np.ascontiguousarray(x2[c * M:(c + 1) * M].T)      # [K, M]
        in_maps.append({"xT": xTc, "wT": wT, "bias": bias_rep})
    return in_maps


def run(in_maps, trace=False, **kw):
    return run_bass_kernel_spmd(
        _get_nc(), in_maps, list(range(N_CORES)), trace=trace, **kw
    )


def kernel(x, weight_2bit, weight_scale, bias):
    res = run(make_in_maps(x, weight_2bit, weight_scale, bias))
    out = np.concatenate([r["out"] for r in res.results], axis=0)
    return np.ascontiguousarray(out.reshape(B, S, N))


# revision 15
# speedup vs baseline: 1.0083x; 1.0022x over previous
"""BitLinear 2-bit quantized linear layer on 8 TRN2 NeuronCores.

Math: reference computes
    a      = clip(max|x| over last dim, EPS)
    out    = ((x/a) @ W_deq^T) * (a*scale) + bias,  W_deq = QUANT_LEVELS[codes]
The per-row absmax normalization cancels exactly (division by `a` then
multiplication by the same `a`), so out == (x @ W_deq^T) * scale + bias.
QUANT_LEVELS[c] = c - 1.5, so W_deq (and W_deq*scale for scale=1) is exactly
representable in bf16. We therefore run a plain bf16 matmul with fp32 PSUM
accumulation and a bias epilogue.

Sharding: data-parallel over the 8192 = 4*2048 (batch*seq) rows; each of the
8 cores computes a [1024, 4096] slice of the output with the full weight.
Host pre-transposes both operands so the device only does DMA + matmul:
  xT [K=4096, M=1024] bf16 per core, wT [K=4096, N=4096] bf16 replicated.
"""

import time

import numpy as np
import ml_dtypes

import concourse.mybir as mybir
from concourse import bacc
from concourse.tile import TileContext
from concourse.bass_utils import run_bass_kernel_spmd

N_CORES = 8
B, S, D_IN, D_OUT = 4, 2048, 4096, 4096
M_TOTAL = B * S              # 8192 rows
M = M_TOTAL // N_CORES       # 1024 rows per core
K = D_IN
N = D_OUT
P = 128                      # partitions
KI = K // P                  # 32 k-tiles
NF = 512                     # psum free dim (one PSUM bank of fp32)
NI = N // NF                 # 8 n-chunks
MI = M // P                  # 8 m-tiles

BF16 = mybir.dt.bfloat16
F32 = mybir.dt.float32


def build(m=M, k=K, n=N):
    ki, mi_n, ni_n = k // P, m // P, n // NF
    nc = bacc.Bacc()
    xT = nc.declare_dram_parameter("xT", [k, m], BF16, isOutput=False)
    wT = nc.declare_dram_parameter("wT", [k, n], BF16, isOutput=False)
    bias = nc.declare_dram_parameter("bias", [P, n], F32, isOutput=False)
    out = nc.declare_dram_parameter("out", [m, n], F32, isOutput=True)

    xT3 = xT[:].rearrange("(a p) m -> p a m", p=P)   # [128, ki, m]
    wT3 = wT[:].rearrange("(a p) n -> p a n", p=P)   # [128, ki, n]

    with TileContext(nc) as tc:
        with (
            tc.tile_pool(name="xpool", bufs=1) as xpool,
            tc.tile_pool(name="bpool", bufs=1) as bpool,
            tc.tile_pool(name="wpool", bufs=2) as wpool,
            tc.tile_pool(name="opool", bufs=6) as opool,
            tc.tile_pool(name="ppool", bufs=8, space="PSUM") as ppool,
        ):
            # x is resident for the whole kernel; the first W chunk and x are
            # loaded interleaved in ki-order pieces so ni=0 matmuls can start
            # after ~1.5 MiB instead of the full 12 MiB. x goes through the
            # ACT DGE ring and w through the SP ring so descriptor generation
            # for the two streams runs in parallel.
            xt = xpool.tile([P, ki, m], BF16, name="xt")
            wg = 8 if ki % 8 == 0 else 1
            kj = ki // wg
            wt0 = wpool.tile([P, ki, NF], BF16, name="wt")
            if wg > 1:
                # smaller leading pieces so the first matmuls unblock sooner
                chunk_sizes = [kj // 2, kj // 2, kj // 2, kj // 2] + [kj] * (wg - 2)
            else:
                chunk_sizes = [kj]
            pos = 0
            for cs in chunk_sizes:
                sl = slice(pos, pos + cs)
                nc.scalar.dma_start(out=xt[:, sl, :], in_=xT3[:, sl, :])
                nc.sync.dma_start(out=wt0[:, sl, :], in_=wT3[:, sl, 0:NF])
                pos += cs
            bias_sb = bpool.tile([P, n], F32, name="bias_sb")
            nc.scalar.dma_start(out=bias_sb[:], in_=bias[:])

            # PE warmup: dummy matmuls on zeroed tiles keep the PE busy while
            # the first data chunks stream in, so the HAM clock-gate reaches
            # 2.4 GHz before the real accumulation starts (saves the ~10 us
            # cold-clock window). Results land in a psum bank that the real
            # ni=0 group overwrites (start=True resets the bank).
            warm_l = bpool.tile([P, P], BF16, name="warm_l")
            warm_r = bpool.tile([P, NF], BF16, name="warm_r")
            nc.vector.memset(warm_l[:], 0.0)
            nc.vector.memset(warm_r[:], 0.0)

            def epilogue(ps, mi, nsl):
                ot = opool.tile([P, NF], F32, name="ot")
                nc.vector.tensor_add(out=ot[:], in0=ps[:], in1=bias_sb[:, nsl])
                nc.sync.dma_start(out=out[mi * P:(mi + 1) * P, nsl], in_=ot[:])

            wt = wt0
            for ni in range(ni_n):
                nsl = slice(ni * NF, (ni + 1) * NF)
                wt_next = None
                if ni + 1 < ni_n:
                    wt_next = wpool.tile([P, ki, NF], BF16, name="wt")
                if ni == 0:
                    # ki-chunk-major over all 8 psum banks: accumulate into
                    # every mi's bank as each ki piece of x/w arrives, so PE
                    # rides right behind the startup DMA stream.
                    pss = [ppool.tile([P, NF], F32, name="ps") for _ in range(mi_n)]
                    for _ in range(22):
                        nc.tensor.matmul(
                            pss[mi_n - 1][:], lhsT=warm_l[:], rhs=warm_r[:],
                            start=True, stop=True,
                        )
                    for g in range(wg):
                        for mi in range(mi_n):
                            for kk in range(g * kj, (g + 1) * kj):
                                nc.tensor.matmul(
                                    pss[mi][:],
                                    lhsT=xt[:, kk, mi * P:(mi + 1) * P],
                                    rhs=wt[:, kk, :],
                                    start=(kk == 0),
                                    stop=(kk == ki - 1),
                                )
                        # delay the ni=1 weight prefetch until the startup
                        # stream is done so they don't race for HBM bandwidth
                        if g == wg - 1 and wt_next is not None:
                            for g2 in range(wg):
                                sl = slice(g2 * kj, (g2 + 1) * kj)
                                nc.sync.dma_start(
                                    out=wt_next[:, sl, :],
                                    in_=wT3[:, sl, NF:2 * NF],
                                )
                    for mi in range(mi_n):
                        epilogue(pss[mi], mi, nsl)
                else:
                    for mi in range(mi_n):
                        ps = ppool.tile([P, NF], F32, name="ps")
                        for kk in range(ki):
                            nc.tensor.matmul(
                                ps[:],
                                lhsT=xt[:, kk, mi * P:(mi + 1) * P],
                                rhs=wt[:, kk, :],
                                start=(kk == 0),
                                stop=(kk == ki - 1),
                            )
                        epilogue(ps, mi, nsl)
                        # spread next-chunk weight DMA issue across the phase
                        if wt_next is not None and mi < wg:
                            sl = slice(mi * kj, (mi + 1) * kj)
                            nc.sync.dma_start(
                                out=wt_next[:, sl, :],
                                in_=wT3[:, sl, (ni + 1) * NF:(ni + 2) * NF],
                            )
                wt = wt_next
    nc.finalize()
    return nc


_NC = None


def _get_nc():
    global _NC
    if _NC is None:
        _NC = build()
    return _NC


def make_in_maps(x, weight_2bit, weight_scale, bias):
    x = np.asarray(x)
    codes = np.asarray(weight_2bit)
    ws = np.float32(np.asarray(weight_scale).reshape(-1)[0])
    b = np.asarray(bias).astype(np.float32)

    w_f = (codes.astype(np.float32) - np.float32(1.5)) * ws      # [N, K]
    wT = np.ascontiguousarray(w_f.T.astype(ml_dtypes.bfloat16))  # [K, N]
    bias_rep = np.ascontiguousarray(np.broadcast_to(b, (P, N)))

    x2 = x.reshape(M_TOTAL, K).astype(ml_dtypes.bfloat16)
    in_maps = []
    for c in range(N_CORES):
        xTc = np.ascontiguousarray(x2[c * M:(c + 1) * M].T)      # [K, M]
        in_maps.append({"xT": xTc, "wT": wT, "bias": bias_rep})
    return in_maps


def run(in_maps, trace=False, **kw):
    # The axon-tunneled devices occasionally fail a fresh process's first
    # execution with NRT_EXEC_UNIT_UNRECOVERABLE; an identical retry succeeds.
    last = None
    for attempt in range(3):
        try:
            return run_bass_kernel_spmd(
                _get_nc(), in_maps, list(range(N_CORES)), trace=trace, **kw
            )
        except Exception as e:
            last = e
            msg = str(e)
            if "UNAVAILABLE" in msg or "unrecoverable" in msg.lower():
                time.sleep(10 * (attempt + 1))
                continue
            raise
    raise last


def kernel(x, weight_2bit, weight_scale, bias):
    res = run(make_in_maps(x, weight_2bit, weight_scale, bias))
    out = np.concatenate([r["out"] for r in res.results], axis=0)
    return np.ascontiguousarray(out.reshape(B, S, N))


# revision 20
# speedup vs baseline: 1.0127x; 1.0043x over previous
"""BitLinear 2-bit quantized linear layer on 8 TRN2 NeuronCores.

Math: reference computes
    a      = clip(max|x| over last dim, EPS)
    out    = ((x/a) @ W_deq^T) * (a*scale) + bias,  W_deq = QUANT_LEVELS[codes]
The per-row absmax normalization cancels exactly (division by `a` then
multiplication by the same `a`), so out == (x @ W_deq^T) * scale + bias.
QUANT_LEVELS[c] = c - 1.5, so W_deq (and W_deq*scale for scale=1) is exactly
representable in bf16. We therefore run a plain bf16 matmul with fp32 PSUM
accumulation and a bias epilogue.

Sharding: data-parallel over the 8192 = 4*2048 (batch*seq) rows; each of the
8 cores computes a [1024, 4096] slice of the output with the full weight.
Host pre-transposes both operands so the device only does DMA + matmul:
  xT [K=4096, M=1024] bf16 per core, wT [K=4096, N=4096] bf16 replicated.
"""

import time

import numpy as np
import ml_dtypes

import concourse.mybir as mybir
from concourse import bacc
from concourse.tile import TileContext
from concourse.bass_utils import run_bass_kernel_spmd

N_CORES = 8
B, S, D_IN, D_OUT = 4, 2048, 4096, 4096
M_TOTAL = B * S              # 8192 rows
M = M_TOTAL // N_CORES       # 1024 rows per core
K = D_IN
N = D_OUT
P = 128                      # partitions
KI = K // P                  # 32 k-tiles
NF = 512                     # psum free dim (one PSUM bank of fp32)
NI = N // NF                 # 8 n-chunks
MI = M // P                  # 8 m-tiles

BF16 = mybir.dt.bfloat16
F32 = mybir.dt.float32


def build(m=M, k=K, n=N):
    ki, mi_n, ni_n = k // P, m // P, n // NF
    nc = bacc.Bacc()
    xT = nc.declare_dram_parameter("xT", [k, m], BF16, isOutput=False)
    wT = nc.declare_dram_parameter("wT", [k, n], BF16, isOutput=False)
    bias = nc.declare_dram_parameter("bias", [P, n], F32, isOutput=False)
    out = nc.declare_dram_parameter("out", [m, n], F32, isOutput=True)

    xT3 = xT[:].rearrange("(a p) m -> p a m", p=P)   # [128, ki, m]
    wT3 = wT[:].rearrange("(a p) n -> p a n", p=P)   # [128, ki, n]

    with TileContext(nc) as tc:
        with (
            tc.tile_pool(name="xpool", bufs=1) as xpool,
            tc.tile_pool(name="bpool", bufs=1) as bpool,
            tc.tile_pool(name="wpool", bufs=2) as wpool,
            tc.tile_pool(name="opool", bufs=6) as opool,
            tc.tile_pool(name="ppool", bufs=8, space="PSUM") as ppool,
        ):
            # x is resident for the whole kernel; the first W chunk and x are
            # loaded interleaved in ki-order pieces so ni=0 matmuls can start
            # after ~1.5 MiB instead of the full 12 MiB. x goes through the
            # ACT DGE ring and w through the SP ring so descriptor generation
            # for the two streams runs in parallel.
            xt = xpool.tile([P, ki, m], BF16, name="xt")
            wg = 8 if ki % 8 == 0 else 1
            kj = ki // wg
            wt0 = wpool.tile([P, ki, NF], BF16, name="wt")
            if wg > 1 and (ki - 4) % kj == 0:
                # smaller leading pieces so the first matmuls unblock sooner
                chunk_sizes = [1, 1, 2] + [kj] * ((ki - 4) // kj)
            else:
                chunk_sizes = [kj] * wg
            assert sum(chunk_sizes) == ki
            pos = 0
            for cs in chunk_sizes:
                sl = slice(pos, pos + cs)
                nc.scalar.dma_start(out=xt[:, sl, :], in_=xT3[:, sl, :])
                nc.sync.dma_start(out=wt0[:, sl, :], in_=wT3[:, sl, 0:NF])
                pos += cs
            bias_sb = bpool.tile([P, n], F32, name="bias_sb")
            nc.scalar.dma_start(out=bias_sb[:], in_=bias[:])

            # PE warmup: dummy matmuls on zeroed tiles keep the PE busy while
            # the first data chunks stream in, so the HAM clock-gate reaches
            # 2.4 GHz before the real accumulation starts (saves the ~10 us
            # cold-clock window). Results land in a psum bank that the real
            # ni=0 group overwrites (start=True resets the bank).
            warm_l = bpool.tile([P, P], BF16, name="warm_l")
            warm_r = bpool.tile([P, NF], BF16, name="warm_r")
            nc.vector.memset(warm_l[:], 0.0)
            nc.vector.memset(warm_r[:], 0.0)

            def epilogue(ps, mi, nsl):
                ot = opool.tile([P, NF], F32, name="ot")
                nc.vector.tensor_add(out=ot[:], in0=ps[:], in1=bias_sb[:, nsl])
                nc.sync.dma_start(out=out[mi * P:(mi + 1) * P, nsl], in_=ot[:])

            wt = wt0
            for ni in range(ni_n):
                nsl = slice(ni * NF, (ni + 1) * NF)
                wt_next = None
                if ni + 1 < ni_n:
                    wt_next = wpool.tile([P, ki, NF], BF16, name="wt")
                if ni == 0:
                    # ki-chunk-major over all 8 psum banks: accumulate into
                    # every mi's bank as each ki piece of x/w arrives, so PE
                    # rides right behind the startup DMA stream.
                    pss = [ppool.tile([P, NF], F32, name="ps") for _ in range(mi_n)]
                    for _ in range(12):
                        nc.tensor.matmul(
                            pss[mi_n - 1][:], lhsT=warm_l[:], rhs=warm_r[:],
                            start=True, stop=True,
                        )
                    cpos = 0
                    for gi, cs in enumerate(chunk_sizes):
                        for mi in range(mi_n):
                            for kk in range(cpos, cpos + cs):
                                nc.tensor.matmul(
                                    pss[mi][:],
                                    lhsT=xt[:, kk, mi * P:(mi + 1) * P],
                                    rhs=wt[:, kk, :],
                                    start=(kk == 0),
                                    stop=(kk == ki - 1),
                                )
                        cpos += cs
                        # delay the ni=1 weight prefetch until the startup
                        # stream is done so they don't race for HBM bandwidth
                        if gi == len(chunk_sizes) - 1 and wt_next is not None:
                            for g2 in range(wg):
                                sl = slice(g2 * kj, (g2 + 1) * kj)
                                nc.sync.dma_start(
                                    out=wt_next[:, sl, :],
                                    in_=wT3[:, sl, NF:2 * NF],
                                )
                    for mi in range(mi_n):
                        epilogue(pss[mi], mi, nsl)
                else:
                    for mi in range(mi_n):
                        last_group = ni == ni_n - 1 and mi == mi_n - 1
                        if last_group:
                            # split the final group into two pipelined halves
                            # so the kernel-tail drain only waits on a short
                            # half-width epilogue chain after the last matmul
                            hf = NF // 2
                            for half in range(2):
                                ps = ppool.tile([P, hf], F32, name="ps")
                                for kk in range(ki):
                                    nc.tensor.matmul(
                                        ps[:],
                                        lhsT=xt[:, kk, mi * P:(mi + 1) * P],
                                        rhs=wt[:, kk, half * hf:(half + 1) * hf],
                                        start=(kk == 0),
                                        stop=(kk == ki - 1),
                                    )
                                hsl = slice(ni * NF + half * hf,
                                            ni * NF + (half + 1) * hf)
                                ot = opool.tile([P, hf], F32, name="ot")
                                nc.vector.tensor_add(
                                    out=ot[:], in0=ps[:], in1=bias_sb[:, hsl])
                                nc.sync.dma_start(
                                    out=out[mi * P:(mi + 1) * P, hsl], in_=ot[:])
                            continue
                        ps = ppool.tile([P, NF], F32, name="ps")
                        for kk in range(ki):
                            nc.tensor.matmul(
                                ps[:],
                                lhsT=xt[:, kk, mi * P:(mi + 1) * P],
                                rhs=wt[:, kk, :],
                                start=(kk == 0),
                                stop=(kk == ki - 1),
                            )
                        epilogue(ps, mi, nsl)
                        # spread next-chunk weight DMA issue across the phase
                        if wt_next is not None and mi < wg:
                            sl = slice(mi * kj, (mi + 1) * kj)
                            nc.sync.dma_start(
                                out=wt_next[:, sl, :],
                                in_=wT3[:, sl, (ni + 1) * NF:(ni + 2) * NF],
                            )
                wt = wt_next
    nc.finalize()
    return nc


_NC = None


def _get_nc():
    global _NC
    if _NC is None:
        _NC = build()
    return _NC


def make_in_maps(x, weight_2bit, weight_scale, bias):
    x = np.asarray(x)
    codes = np.asarray(weight_2bit)
    ws = np.float32(np.asarray(weight_scale).reshape(-1)[0])
    b = np.asarray(bias).astype(np.float32)

    w_f = (codes.astype(np.float32) - np.float32(1.5)) * ws      # [N, K]
    wT = np.ascontiguousarray(w_f.T.astype(ml_dtypes.bfloat16))  # [K, N]
    bias_rep = np.ascontiguousarray(np.broadcast_to(b, (P, N)))

    x2 = x.reshape(M_TOTAL, K).astype(ml_dtypes.bfloat16)
    in_maps = []
    for c in range(N_CORES):
        xTc = np.ascontiguousarray(x2[c * M:(c + 1) * M].T)      # [K, M]
        in_maps.append({"xT": xTc, "wT": wT, "bias": bias_rep})
    return in_maps


def run(in_maps, trace=False, **kw):
    # The axon-tunneled devices occasionally fail a fresh process's first
    # execution with NRT_EXEC_UNIT_UNRECOVERABLE; an identical retry succeeds.
    last = None
    for attempt in range(3):
        try:
            return run_bass_kernel_spmd(
                _get_nc(), in_maps, list(range(N_CORES)), trace=trace, **kw
            )
        except Exception as e:
            last = e
            msg = str(e)
            if "UNAVAILABLE" in msg or "unrecoverable" in msg.lower():
                # the failure is sticky in the PJRT client: drop the backend
                # so the next attempt re-opens the devices
                try:
                    import jax

                    jax.clear_caches()
                    import jax.extend.backend

                    jax.extend.backend.clear_backends()
                except Exception:
                    pass
                time.sleep(10 * (attempt + 1))
                continue
            raise
    raise last


def kernel(x, weight_2bit, weight_scale, bias):
    res = run(make_in_maps(x, weight_2bit, weight_scale, bias))
    out = np.concatenate([r["out"] for r in res.results], axis=0)
    return np.ascontiguousarray(out.reshape(B, S, N))
